# revision 3
# baseline (speedup 1.0000x reference)
"""AnomalyAttention (banded, |i-j| < 64) Bass kernel for 8 TRN2 NeuronCores.

Sharding: B*H = 16 (b,h) pairs, 2 per core (data/head parallel, no collectives).
Each core computes, per pair, the banded softmax attention matrix ("series",
dense [L, L] with zeros outside the band) and the attention output V.

Device-side layout per pair:
  qt, kt : [E=64, L=2048]  (host pre-transposes so E is the contraction/partition dim)
  v      : [L, 2048->shifted tiles]  natural [L, E], loaded 64-shifted so the
           two PV contraction chunks per row-block align to tile partitions.
Outputs per pair:
  series : [L, L] dense f32 (band written from compute, rest zeros)
  vout   : [L, E]
"""

import numpy as np

B, L, H, E = 2, 2048, 8, 64
N_CORES = 8
PAIRS = 2          # (b,h) pairs per core
NBLK = L // 128    # 16 row blocks per pair
NEG = -1.0e30
SCALE = 0.125      # 1/sqrt(E)

_CACHE = {}


def _windows():
    wins = []
    for n in range(NBLK):
        if n == 0:
            wins.append((0, 192))
        elif n == NBLK - 1:
            wins.append((L - 192, 192))
        else:
            wins.append((128 * n - 64, 256))
    return wins


def _stale_window(pair, n, wins):
    """Window of this D tile's previous use (must be re-zeroed), or None."""
    if pair == 0 and n < 3:
        return None
    if n >= 3:
        return wins[n - 3]
    # previous pair's last use of tile (n % 3): largest m <= 15 with m % 3 == n
    m = max(mm for mm in range(NBLK) if mm % 3 == n % 3)
    return wins[m]


def _pv_chunks(n):
    """[(c0, cw, vm, vp0)]: P cols [c0, c0+cw) x vsh tile vm partitions [vp0, vp0+cw)."""
    if n == 0:
        return [(0, 64, 0, 0), (64, 128, 1, 0)]
    if n == NBLK - 1:
        return [(0, 128, NBLK - 1, 0), (128, 64, NBLK, 0)]
    return [(0, 128, n, 0), (128, 128, n + 1, 0)]


def _build():
    from contextlib import ExitStack

    import concourse.bacc as bacc
    import concourse.tile as tile
    from concourse import mybir
    from concourse.masks import make_identity

    f32 = mybir.dt.float32
    ge = mybir.AluOpType.is_ge

    nc = bacc.Bacc()
    qt_h = nc.declare_dram_parameter("qt", [PAIRS, E, L], f32, isOutput=False)
    kt_h = nc.declare_dram_parameter("kt", [PAIRS, E, L], f32, isOutput=False)
    v_h = nc.declare_dram_parameter("v", [PAIRS, L, E], f32, isOutput=False)
    ser_h = nc.declare_dram_parameter("series", [PAIRS, L, L], f32, isOutput=True)
    vout_h = nc.declare_dram_parameter("vout", [PAIRS, L, E], f32, isOutput=True)

    wins = _windows()

    with ExitStack() as ctx:
        tc = ctx.enter_context(tile.TileContext(nc))
        singles = ctx.enter_context(tc.tile_pool(name="singles", bufs=1))
        io = ctx.enter_context(tc.tile_pool(name="io", bufs=2))
        work = ctx.enter_context(tc.tile_pool(name="work", bufs=3))
        ptp = ctx.enter_context(tc.tile_pool(name="ptp", bufs=2))
        psum = ctx.enter_context(tc.tile_pool(name="psum", bufs=2, space="PSUM"))

        identity = singles.tile([128, 128], f32, tag="ident")
        make_identity(nc, identity[:])

        # additive band masks: 0 in-band, NEG out-of-band
        # middle blocks (window starts at r0-64): valid iff p+1 <= c <= p+127
        mask_mid = singles.tile([128, 256], f32, tag="mmid")
        nc.gpsimd.memset(mask_mid[:], 0.0)
        nc.gpsimd.affine_select(
            out=mask_mid[:], in_=mask_mid[:], compare_op=ge, fill=NEG,
            base=-1, channel_multiplier=-1, pattern=[[1, 256]])  # c - p - 1 >= 0
        nc.gpsimd.affine_select(
            out=mask_mid[:], in_=mask_mid[:], compare_op=ge, fill=NEG,
            base=127, channel_multiplier=1, pattern=[[-1, 256]])  # p + 127 - c >= 0
        # first block (window starts at 0): valid iff p-63 <= c <= p+63
        mask_first = singles.tile([128, 192], f32, tag="mfirst")
        nc.gpsimd.memset(mask_first[:], 0.0)
        nc.gpsimd.affine_select(
            out=mask_first[:], in_=mask_first[:], compare_op=ge, fill=NEG,
            base=63, channel_multiplier=-1, pattern=[[1, 192]])  # c - p + 63 >= 0
        nc.gpsimd.affine_select(
            out=mask_first[:], in_=mask_first[:], compare_op=ge, fill=NEG,
            base=63, channel_multiplier=1, pattern=[[-1, 192]])  # p + 63 - c >= 0

        # 3 round-robin dense row-block tiles; zeros persist outside the band
        dts = [
            singles.tile([128, L], f32, tag=f"dense{i}", name=f"dense{i}")
            for i in range(3)
        ]
        for d in dts:
            nc.vector.memset(d[:], 0.0)

        for pair in range(PAIRS):
            qt_t = io.tile([E, L], f32, tag="qt")
            kt_t = io.tile([E, L], f32, tag="kt")
            vsh_t = io.tile([128, NBLK + 1, E], f32, tag="vsh")
            vout_t = io.tile([128, NBLK, E], f32, tag="vout")

            nc.scalar.dma_start(out=qt_t[:], in_=qt_h[pair])
            nc.scalar.dma_start(out=kt_t[:], in_=kt_h[pair])
            # shifted V: tile m in [1,15] holds rows 128m-64+p (p in [0,128));
            # tile 0 holds rows [0,64) at partitions [0,64); tile 16 rows [1984,2048).
            vr = v_h[pair].rearrange("(m p) e -> p m e", p=128)
            nc.scalar.dma_start(out=vsh_t[0:64, 0, :], in_=vr[0:64, 0, :])
            nc.scalar.dma_start(out=vsh_t[0:64, 1:NBLK, :], in_=vr[64:128, 0:NBLK - 1, :])
            nc.scalar.dma_start(out=vsh_t[64:128, 1:NBLK, :], in_=vr[0:64, 1:NBLK, :])
            nc.scalar.dma_start(out=vsh_t[0:64, NBLK, :], in_=vr[64:128, NBLK - 1, :])

            for n in range(NBLK):
                w0, W = wins[n]
                d = dts[n % 3]

                stale = _stale_window(pair, n, wins)
                if stale is not None:
                    sw0, sW = stale
                    nc.gpsimd.memset(d[:, sw0:sw0 + sW], 0.0)

                # banded scores S = Q_blk @ K_win^T  ([128, W] PSUM)
                s_ps = psum.tile([128, 256], f32, tag="s")
                nc.tensor.matmul(
                    s_ps[:, :W],
                    qt_t[:, n * 128:(n + 1) * 128],
                    kt_t[:, w0:w0 + W],
                )

                if n == 0:
                    mask_ap = mask_first[:]
                else:
                    mask_ap = mask_mid[:] if W == 256 else mask_mid[:, 0:192]

                # sm = S + mask  (scores/8 ~ N(0,1): exp never overflows, so no
                # row-max subtraction needed; masked entries -> exp(-1.25e29)=0)
                sm = work.tile([128, 256], f32, tag="sm")
                nc.vector.tensor_add(sm[:, :W], s_ps[:, :W], mask_ap)

                # e = exp(SCALE * sm) ; den = rowsum(e)
                esb = work.tile([128, 256], f32, tag="esb")
                den = work.tile([128, 1], f32, tag="den")
                nc.scalar.activation(
                    out=esb[:, :W], in_=sm[:, :W],
                    func=mybir.ActivationFunctionType.Exp,
                    bias=0.0, scale=SCALE, accum_out=den[:],
                )
                rec = work.tile([128, 1], f32, tag="rec")
                nc.vector.reciprocal(rec[:], den[:])

                # normalized P written straight into the dense tile's window
                nc.vector.tensor_scalar_mul(d[:, w0:w0 + W], esb[:, :W], rec[:, 0:1])

                # O = P @ V via 2 chunks: lhsT = P^T chunk (PE transpose), rhs = shifted V
                o_ps = psum.tile([128, E], f32, tag="o")
                chunks = _pv_chunks(n)
                for ci, (c0, cw, vm, vp0) in enumerate(chunks):
                    pt_ps = psum.tile([128, 128], f32, tag=f"pt{ci}")
                    nc.tensor.transpose(
                        pt_ps[0:cw, :], d[:, w0 + c0:w0 + c0 + cw], identity[:])
                    pt_sb = ptp.tile([128, 128], f32, tag=f"ptsb{ci}")
                    nc.vector.tensor_copy(pt_sb[0:cw, :], pt_ps[0:cw, :])
                    nc.tensor.matmul(
                        o_ps[:],
                        pt_sb[0:cw, :],
                        vsh_t[vp0:vp0 + cw, vm, :],
                        start=(ci == 0),
                        stop=(ci == len(chunks) - 1),
                    )
                nc.scalar.copy(vout_t[:, n, :], o_ps[:])

                # dense row-block out (1 MiB contiguous in DRAM)
                nc.sync.dma_start(
                    out=ser_h[pair, n * 128:(n + 1) * 128, :], in_=d[:])

            vw = vout_h[pair].rearrange("(n p) e -> p n e", p=128)
            nc.sync.dma_start(out=vw[:], in_=vout_t[:])

    nc.compile()
    return nc


def _get_nc():
    if "nc" not in _CACHE:
        _CACHE["nc"] = _build()
    return _CACHE["nc"]


def _shard_inputs(queries, keys, values):
    """-> list of 8 in_maps; pair index = b*H + h, core c gets pairs [2c, 2c+1]."""
    q = np.ascontiguousarray(np.asarray(queries, dtype=np.float32))
    k = np.ascontiguousarray(np.asarray(keys, dtype=np.float32))
    v = np.ascontiguousarray(np.asarray(values, dtype=np.float32))
    # [B, L, H, E] -> [B*H, E, L] for q/k, [B*H, L, E] for v
    qt = np.ascontiguousarray(q.transpose(0, 2, 3, 1).reshape(B * H, E, L))
    kt = np.ascontiguousarray(k.transpose(0, 2, 3, 1).reshape(B * H, E, L))
    vn = np.ascontiguousarray(v.transpose(0, 2, 1, 3).reshape(B * H, L, E))
    in_maps = []
    for c in range(N_CORES):
        sl = slice(2 * c, 2 * c + 2)
        in_maps.append({
            "qt": np.ascontiguousarray(qt[sl]),
            "kt": np.ascontiguousarray(kt[sl]),
            "v": np.ascontiguousarray(vn[sl]),
        })
    return in_maps


def _run(queries, keys, values, trace=False, **trace_kwargs):
    from concourse.bass_utils import run_bass_kernel_spmd

    nc = _get_nc()
    in_maps = _shard_inputs(queries, keys, values)
    res = run_bass_kernel_spmd(
        nc, in_maps, list(range(N_CORES)), trace=trace, **trace_kwargs)

    v_full = np.empty((B, L, H, E), dtype=np.float32)
    series = np.empty((B * H, L, L), dtype=np.float32)
    for c in range(N_CORES):
        out = res.results[c]
        series[2 * c:2 * c + 2] = out["series"]
        for p in range(PAIRS):
            idx = 2 * c + p
            v_full[idx // H, :, idx % H, :] = out["vout"][p]
    return v_full, series.reshape(B, H, L, L), res


def kernel(queries, keys, values, sigma=None, attn_mask=None, **_unused):
    v_full, series, _ = _run(queries, keys, values, trace=False)
    return (v_full, series)


# revision 4
# speedup vs baseline: 2.0405x; 2.0405x over previous
"""AnomalyAttention (banded, |i-j| < 64) Bass kernel for 8 TRN2 NeuronCores.

Sharding: B*H = 16 (b,h) pairs, 2 per core (data/head parallel, no collectives).
Each core computes, per pair, the banded softmax attention matrix ("series",
dense [L, L] where only the |i-j| < 64 band is nonzero) and the attention
output V.

The runtime pre-zeros ExternalOutput buffers (run_bass_via_pjrt donates
zero-initialized buffers; kernels that don't write every element rely on
that), so the device writes ONLY the band windows of `series` - the dense
zeros come from the donated output buffer.

Device-side layout per pair:
  qt, kt : [E=64, L=2048]  (host pre-transposes so E is the contraction dim)
  v      : [L, E] natural, loaded 64-shifted so the two PV contraction chunks
           per row-block align to tile partition boundaries.
"""

import numpy as np

B, L, H, E = 2, 2048, 8, 64
N_CORES = 8
PAIRS = 2          # (b,h) pairs per core
NBLK = L // 128    # 16 row blocks per pair
NEG = -1.0e30
SCALE = 0.125      # 1/sqrt(E)
COMPUTE = "bf16"   # "bf16" or "f32" matmul operand dtype

_CACHE = {}


def _windows():
    wins = []
    for n in range(NBLK):
        if n == 0:
            wins.append((0, 192))
        elif n == NBLK - 1:
            wins.append((L - 192, 192))
        else:
            wins.append((128 * n - 64, 256))
    return wins


def _pv_chunks(n):
    """[(c0, cw, vm)]: P cols [c0, c0+cw) contract with vsh tile vm parts [0, cw)."""
    if n == 0:
        return [(0, 64, 0), (64, 128, 1)]
    if n == NBLK - 1:
        return [(0, 128, NBLK - 1), (128, 64, NBLK)]
    return [(0, 128, n), (128, 128, n + 1)]


def _build():
    from contextlib import ExitStack

    import concourse.bacc as bacc
    import concourse.tile as tile
    from concourse import mybir
    from concourse.masks import make_identity

    f32 = mybir.dt.float32
    cdt = mybir.dt.bfloat16 if COMPUTE == "bf16" else f32
    ge = mybir.AluOpType.is_ge

    nc = bacc.Bacc()
    qt_h = nc.declare_dram_parameter("qt", [PAIRS, E, L], f32, isOutput=False)
    kt_h = nc.declare_dram_parameter("kt", [PAIRS, E, L], f32, isOutput=False)
    v_h = nc.declare_dram_parameter("v", [PAIRS, L, E], f32, isOutput=False)
    ser_h = nc.declare_dram_parameter("series", [PAIRS, L, L], f32, isOutput=True)
    vout_h = nc.declare_dram_parameter("vout", [PAIRS, L, E], f32, isOutput=True)

    wins = _windows()

    with ExitStack() as ctx:
        tc = ctx.enter_context(tile.TileContext(nc))
        singles = ctx.enter_context(tc.tile_pool(name="singles", bufs=1))
        io = ctx.enter_context(tc.tile_pool(name="io", bufs=2))
        work = ctx.enter_context(tc.tile_pool(name="work", bufs=4))
        ptp = ctx.enter_context(tc.tile_pool(name="ptp", bufs=2))
        psum = ctx.enter_context(tc.tile_pool(name="psum", bufs=2, space="PSUM"))

        identity = singles.tile([128, 128], f32, tag="ident")
        make_identity(nc, identity[:])

        # additive band masks: 0 in-band, NEG out-of-band
        # middle blocks (window starts at r0-64): valid iff p+1 <= c <= p+127
        mask_mid = singles.tile([128, 256], f32, tag="mmid")
        nc.gpsimd.memset(mask_mid[:], 0.0)
        nc.gpsimd.affine_select(
            out=mask_mid[:], in_=mask_mid[:], compare_op=ge, fill=NEG,
            base=-1, channel_multiplier=-1, pattern=[[1, 256]])  # c - p - 1 >= 0
        nc.gpsimd.affine_select(
            out=mask_mid[:], in_=mask_mid[:], compare_op=ge, fill=NEG,
            base=127, channel_multiplier=1, pattern=[[-1, 256]])  # p + 127 - c >= 0
        # first block (window starts at 0): valid iff p-63 <= c <= p+63
        mask_first = singles.tile([128, 192], f32, tag="mfirst")
        nc.gpsimd.memset(mask_first[:], 0.0)
        nc.gpsimd.affine_select(
            out=mask_first[:], in_=mask_first[:], compare_op=ge, fill=NEG,
            base=63, channel_multiplier=-1, pattern=[[1, 192]])  # c - p + 63 >= 0
        nc.gpsimd.affine_select(
            out=mask_first[:], in_=mask_first[:], compare_op=ge, fill=NEG,
            base=63, channel_multiplier=1, pattern=[[-1, 192]])  # p + 63 - c >= 0

        for pair in range(PAIRS):
            qt_t = io.tile([E, L], cdt, tag="qt")
            kt_t = io.tile([E, L], cdt, tag="kt")
            vsh_t = io.tile([128, NBLK + 1, E], cdt, tag="vsh")
            vout_t = io.tile([128, NBLK, E], f32, tag="vout")

            if COMPUTE == "bf16":
                ldeng = nc.gpsimd  # SWDGE casts f32 -> bf16 during DMA
            else:
                ldeng = nc.scalar
            ldeng.dma_start(out=qt_t[:], in_=qt_h[pair])
            ldeng.dma_start(out=kt_t[:], in_=kt_h[pair])
            # shifted V: tile m in [1,15] holds rows 128m-64+p (p in [0,128));
            # tile 0 holds rows [0,64), tile 16 rows [1984,2048), at parts [0,64).
            vr = v_h[pair].rearrange("(m p) e -> p m e", p=128)
            ldeng.dma_start(out=vsh_t[0:64, 0, :], in_=vr[0:64, 0, :])
            ldeng.dma_start(out=vsh_t[0:64, 1:NBLK, :], in_=vr[64:128, 0:NBLK - 1, :])
            ldeng.dma_start(out=vsh_t[64:128, 1:NBLK, :], in_=vr[0:64, 1:NBLK, :])
            ldeng.dma_start(out=vsh_t[0:64, NBLK, :], in_=vr[64:128, NBLK - 1, :])

            for n in range(NBLK):
                w0, W = wins[n]

                # banded scores S = Q_blk @ K_win^T  ([128, W] PSUM)
                s_ps = psum.tile([128, 256], f32, tag="s")
                nc.tensor.matmul(
                    s_ps[:, :W],
                    qt_t[:, n * 128:(n + 1) * 128],
                    kt_t[:, w0:w0 + W],
                )

                if n == 0:
                    mask_ap = mask_first[:]
                else:
                    mask_ap = mask_mid[:] if W == 256 else mask_mid[:, 0:192]

                # sm = S + mask  (scores/8 ~ N(0,1): exp never overflows, so no
                # row-max subtraction needed; masked -> exp(-1.25e29) = 0)
                sm = work.tile([128, 256], f32, tag="sm")
                nc.vector.tensor_add(sm[:, :W], s_ps[:, :W], mask_ap)

                # e = exp(SCALE * sm) ; den = rowsum(e)
                esb = work.tile([128, 256], f32, tag="esb")
                den = work.tile([128, 1], f32, tag="den")
                nc.scalar.activation(
                    out=esb[:, :W], in_=sm[:, :W],
                    func=mybir.ActivationFunctionType.Exp,
                    bias=0.0, scale=SCALE, accum_out=den[:],
                )
                rec = work.tile([128, 1], f32, tag="rec")
                nc.vector.reciprocal(rec[:], den[:])

                # normalized band P
                pw = work.tile([128, 256], f32, tag="pw")
                nc.vector.tensor_scalar_mul(pw[:, :W], esb[:, :W], rec[:, 0:1])

                # O = P @ V via 2 chunks: lhsT = P^T chunk (PE transpose), rhs = shifted V
                o_ps = psum.tile([128, E], f32, tag="o")
                chunks = _pv_chunks(n)
                for ci, (c0, cw, vm) in enumerate(chunks):
                    pt_ps = psum.tile([128, 128], f32, tag=f"pt{ci}")
                    nc.tensor.transpose(
                        pt_ps[0:cw, :], pw[:, c0:c0 + cw], identity[:])
                    pt_sb = ptp.tile([128, 128], cdt, tag=f"ptsb{ci}")
                    nc.vector.tensor_copy(pt_sb[0:cw, :], pt_ps[0:cw, :])
                    nc.tensor.matmul(
                        o_ps[:],
                        pt_sb[0:cw, :],
                        vsh_t[0:cw, vm, :],
                        start=(ci == 0),
                        stop=(ci == len(chunks) - 1),
                    )
                nc.scalar.copy(vout_t[:, n, :], o_ps[:])

                # band-window write; the rest of `series` stays host-zeroed
                nc.sync.dma_start(
                    out=ser_h[pair, n * 128:(n + 1) * 128, w0:w0 + W],
                    in_=pw[:, :W])

            vw = vout_h[pair].rearrange("(n p) e -> p n e", p=128)
            nc.sync.dma_start(out=vw[:], in_=vout_t[:])

    nc.compile()
    return nc


def _get_nc():
    if "nc" not in _CACHE:
        _CACHE["nc"] = _build()
    return _CACHE["nc"]


def _shard_inputs(queries, keys, values):
    """-> list of 8 in_maps; pair index = b*H + h, core c gets pairs [2c, 2c+1]."""
    q = np.ascontiguousarray(np.asarray(queries, dtype=np.float32))
    k = np.ascontiguousarray(np.asarray(keys, dtype=np.float32))
    v = np.ascontiguousarray(np.asarray(values, dtype=np.float32))
    # [B, L, H, E] -> [B*H, E, L] for q/k, [B*H, L, E] for v
    qt = np.ascontiguousarray(q.transpose(0, 2, 3, 1).reshape(B * H, E, L))
    kt = np.ascontiguousarray(k.transpose(0, 2, 3, 1).reshape(B * H, E, L))
    vn = np.ascontiguousarray(v.transpose(0, 2, 1, 3).reshape(B * H, L, E))
    in_maps = []
    for c in range(N_CORES):
        sl = slice(2 * c, 2 * c + 2)
        in_maps.append({
            "qt": np.ascontiguousarray(qt[sl]),
            "kt": np.ascontiguousarray(kt[sl]),
            "v": np.ascontiguousarray(vn[sl]),
        })
    return in_maps


def _run(queries, keys, values, trace=False, **trace_kwargs):
    from concourse.bass_utils import run_bass_kernel_spmd

    nc = _get_nc()
    in_maps = _shard_inputs(queries, keys, values)
    res = run_bass_kernel_spmd(
        nc, in_maps, list(range(N_CORES)), trace=trace, **trace_kwargs)

    v_full = np.empty((B, L, H, E), dtype=np.float32)
    series = np.empty((B * H, L, L), dtype=np.float32)
    for c in range(N_CORES):
        out = res.results[c]
        series[2 * c:2 * c + 2] = out["series"]
        for p in range(PAIRS):
            idx = 2 * c + p
            v_full[idx // H, :, idx % H, :] = out["vout"][p]
    return v_full, series.reshape(B, H, L, L), res


def kernel(queries, keys, values, sigma=None, attn_mask=None, **_unused):
    v_full, series, _ = _run(queries, keys, values, trace=False)
    return (v_full, series)
